# revision 15
# baseline (speedup 1.0000x reference)
"""Fused self-attention kernel for Trainium2 (8 NeuronCores, batch-parallel).

Computes, for X of shape (8, 4096, 64):
    out[b] = softmax(X[b] @ X[b].T, axis=-1) @ X[b]
with one batch per NeuronCore.

Per-core algorithm (flash-style, fully on-chip):
  - XTdup (128, 4096) bf16: X^T replicated on partition halves 0-63 and
    64-127 (built with paired PE transposes into col groups 0/64), so the
    K=64 S^T matmuls can be row-packed pairwise via tile_position and run
    two-at-a-time on the PE array.
  - X_ext (4096, 65) = [X | ones] in float32r (PV stationary operand).
  - Per 512-query block, in groups of 3 key-chunks (128 keys each):
      S^T chunks = XT[keys].T @ XT[:, queries]     (bf16, PSUM 3 banks)
      P^T = exp(S^T - 32)                          (one 1536-wide ACTIVATE)
      Y^T_ext += X_ext[keys].T @ P^T               (f32r, PSUM-accumulated
                                                    across the whole block)
    The ones column of X_ext makes row 64 the softmax denominator.
  - PE transposes Y^T_ext back, DVE divides by the denominator, DMA out.
  The group pipeline is flattened across query blocks: S^T emission runs
  two groups ahead of exp/PV so ScalarE (the bottleneck) never starves.

softmax(S) == softmax(S - 32) exactly; the global shift keeps exp within
fp32 range (row maxima of S lie in [29, 111] for unit-normal X).

PSUM budget: S^T double-buffer 2x3 banks + Y accumulator 1 + transpose 1.
"""

import sys

for _p in ("/opt/trn_rl_repo",):
    if _p not in sys.path:
        sys.path.insert(0, _p)

from contextlib import ExitStack

import numpy as np

import concourse.bass as bass
import concourse.tile as tile
from concourse import bacc, mybir
from concourse import bass_utils
from concourse.masks import make_identity

B, S, D = 8, 4096, 64
SHIFT = 32.0
QB = 512  # queries per block
JC = 128  # keys per chunk
GROUP = 3  # key chunks per exp group (PSUM banks per S^T buffer)
N_JC = S // JC  # 32
N_QB = S // QB  # 8

F32 = mybir.dt.float32
F32R = mybir.dt.float32r
BF16 = mybir.dt.bfloat16


def _body(ctx: ExitStack, tc: tile.TileContext, out: bass.AP, x: bass.AP):
    nc = tc.nc

    singles = ctx.enter_context(tc.tile_pool(name="singles", bufs=1))
    pt_pool = ctx.enter_context(tc.tile_pool(name="pt", bufs=3))
    ysb_pool = ctx.enter_context(tc.tile_pool(name="ysb", bufs=2))
    yout_pool = ctx.enter_context(tc.tile_pool(name="yout", bufs=4))
    st_ps = ctx.enter_context(tc.tile_pool(name="st", bufs=2, space="PSUM"))
    yacc_ps = ctx.enter_context(tc.tile_pool(name="yacc", bufs=1, space="PSUM"))
    ytr_ps = ctx.enter_context(tc.tile_pool(name="ytr", bufs=1, space="PSUM"))

    idf32 = singles.tile([D + 1, D + 1], F32)
    make_identity(nc, idf32)
    idbf = singles.tile([128, 128], BF16)
    make_identity(nc, idbf)

    bias = singles.tile([128, 1], F32)
    nc.vector.memset(bias, -SHIFT)

    xext = singles.tile([128, N_JC, D + 1], F32R)
    ones = singles.tile([128, N_JC], F32)
    nc.vector.memset(ones, 1.0)
    nc.vector.tensor_copy(xext[:, :, D], ones)

    xtdup = singles.tile([128, S], BF16)

    # Input phase: 4 chunks per unit. Per-unit DMAs alternate between the
    # sync and gpsimd DGE rings so loads land in parallel. Each slab is
    # converted to f32r (X_ext) and bf16, then transposed with paired PE
    # transposes (col groups 0/64) into a PSUM tile and copied into XTdup.
    # The transpose tiles rotate 3-deep through the ytr, yacc, and one st
    # slot, all idle until the steady-state pipeline starts.
    xld_pool = ctx.enter_context(tc.tile_pool(name="xld", bufs=8))

    def emit_input_unit(u):
        xld = xld_pool.tile([128, 4, D], F32, tag="xld", name="xld")
        src = x[u * 512 : (u + 1) * 512, :].rearrange("(c p) d -> p c d", p=128)
        nc.sync.dma_start(xld, src)
        nc.vector.tensor_copy(xext[:, 4 * u : 4 * u + 4, 0:D], xld)
        xbf = xld_pool.tile([128, 4, D], BF16, tag="xbf", name="xbf")
        nc.vector.tensor_copy(xbf, xld)
        pool, tag = [(ytr_ps, "ytr"), (st_ps, "st")][u % 2]
        ptr = pool.tile([128, 4, 128], BF16, tag=tag, name="ptr")
        for c in range(4):
            nc.tensor.transpose(
                ptr[0:64, c, :], xbf[:, c, :], idbf, tile_position=(0, 0)
            )
            nc.tensor.transpose(
                ptr[64:128, c, :], xbf[:, c, :], idbf, tile_position=(0, 64)
            )
        nc.vector.tensor_copy(
            xtdup[:, u * 512 : (u + 1) * 512].rearrange("p (c j) -> p c j", c=4), ptr
        )

    # Global flattened group schedule.
    groups = []  # (qb, [chunks])
    for qb in range(N_QB):
        lo = 0
        while lo < N_JC:
            groups.append((qb, list(range(lo, min(lo + GROUP, N_JC)))))
            lo += GROUP
    n_g = len(groups)

    def emit_st(i):
        qb, chunks = groups[i]
        st = st_ps.tile([128, GROUP, QB], F32, tag="st")
        q0 = qb * QB
        for ci, jc in enumerate(chunks):
            half = jc % 2
            rows = slice(64 * half, 64 * half + 64)
            nc.tensor.matmul(
                st[:, ci, :],
                xtdup[rows, jc * JC : (jc + 1) * JC],
                xtdup[rows, q0 : q0 + QB],
                start=True,
                stop=True,
                tile_position=(64 * half, 0),
            )
        return st

    def emit_exp(st, i):
        w = len(groups[i][1])
        pt = pt_pool.tile([128, GROUP, QB], F32R, tag="pt")
        nc.scalar.activation(
            pt[:, 0:w, :],
            st[:, 0:w, :],
            mybir.ActivationFunctionType.Exp,
            bias=bias,
            scale=1.0,
        )
        return pt

    def emit_pv(i, pt, yacc):
        for ci, jc in enumerate(groups[i][1]):
            nc.tensor.matmul(
                yacc,
                xext[:, jc, :],
                pt[:, ci, :],
                start=(jc == 0),
                stop=(jc == N_JC - 1),
            )

    def emit_epilogue(qb, yacc):
        ysb = ysb_pool.tile([D + 1, QB], F32, tag="ysb")
        for c in range(QB // 128):
            cs = slice(c * 128, (c + 1) * 128)
            nc.vector.tensor_copy(ysb[:, cs], yacc[:, cs])
            ytr = ytr_ps.tile([128, D + 1], F32, tag="ytr", name="ytr")
            nc.tensor.transpose(ytr, ysb[:, cs], idf32)
            rinv = yout_pool.tile([128, 1], F32, tag="rinv")
            nc.vector.reciprocal(rinv, ytr[:, D : D + 1])
            yo = yout_pool.tile([128, D], F32, tag="yo")
            nc.vector.tensor_scalar_mul(yo, ytr[:, 0:D], rinv)
            eng = nc.sync if c % 2 == 0 else nc.gpsimd
            eng.dma_start(out[qb * QB + c * 128 : qb * QB + (c + 1) * 128, :], yo)

    n_units = N_JC // 4
    units_emitted = 0

    def ensure_units(n):
        nonlocal units_emitted
        while units_emitted < min(n, n_units):
            emit_input_unit(units_emitted)
            units_emitted += 1

    def units_needed(i):
        qb, chunks = groups[i]
        hi = max(chunks[-1], (qb + 1) * (QB // JC) - 1)
        return hi // 4 + 1

    ensure_units(units_needed(0))
    st_tiles = {0: emit_st(0)}
    ensure_units(units_needed(1))
    st_tiles[1] = emit_st(1)
    yaccs = {}
    for i in range(n_g):
        qb, chunks = groups[i]
        if qb not in yaccs:
            yaccs[qb] = yacc_ps.tile([D + 1, QB], F32, tag="yacc", name="yacc")
        pt = emit_exp(st_tiles.pop(i), i)
        if i + 2 < n_g:
            ensure_units(max(units_needed(i + 2), units_emitted + 2))
            st_tiles[i + 2] = emit_st(i + 2)
        emit_pv(i, pt, yaccs[qb])
        if chunks[-1] == N_JC - 1:
            emit_epilogue(qb, yaccs.pop(qb))


def build():
    nc = bacc.Bacc("TRN2", target_bir_lowering=False, debug=False, num_devices=B)
    x = nc.dram_tensor("X", (S, D), F32, kind="ExternalInput").ap()
    out = nc.dram_tensor("out", (S, D), F32, kind="ExternalOutput").ap()
    with tile.TileContext(nc) as tc:
        with ExitStack() as ctx:
            _body(ctx, tc, out, x)
    nc.compile()
    return nc


_NC = None


def run(X: np.ndarray, trace: bool = False, tmpdir: str | None = None):
    global _NC
    if _NC is None:
        _NC = build()
    X = np.asarray(X, dtype=np.float32)
    in_maps = [{"X": np.ascontiguousarray(X[b])} for b in range(B)]
    res = bass_utils.run_bass_kernel_spmd(
        _NC, in_maps, core_ids=list(range(B)), trace=trace, tmpdir=tmpdir
    )
    out = np.stack([res.results[b]["out"] for b in range(B)], axis=0).astype(np.float32)
    return out, res


def kernel(X: np.ndarray) -> np.ndarray:
    out, _ = run(X, trace=False)
    return out


# revision 16
# speedup vs baseline: 1.0055x; 1.0055x over previous
"""Fused self-attention kernel for Trainium2 (8 NeuronCores, batch-parallel).

Computes, for X of shape (8, 4096, 64):
    out[b] = softmax(X[b] @ X[b].T, axis=-1) @ X[b]
with one batch per NeuronCore.

Per-core algorithm (flash-style, fully on-chip):
  - XTdup (128, 4096) bf16: X^T replicated on partition halves 0-63 and
    64-127 (built with paired PE transposes into col groups 0/64), so the
    K=64 S^T matmuls can be row-packed pairwise via tile_position and run
    two-at-a-time on the PE array.
  - X_ext (4096, 65) = [X | ones] in float32r (PV stationary operand).
  - Per 512-query block, in groups of 3 key-chunks (128 keys each):
      S^T chunks = XT[keys].T @ XT[:, queries]     (bf16, PSUM 3 banks)
      P^T = exp(S^T - 32)                          (one 1536-wide ACTIVATE)
      Y^T_ext += X_ext[keys].T @ P^T               (f32r, PSUM-accumulated
                                                    across the whole block)
    The ones column of X_ext makes row 64 the softmax denominator.
  - PE transposes Y^T_ext back, DVE divides by the denominator, DMA out.
  The group pipeline is flattened across query blocks: S^T emission runs
  two groups ahead of exp/PV so ScalarE (the bottleneck) never starves.

softmax(S) == softmax(S - 32) exactly; the global shift keeps exp within
fp32 range (row maxima of S lie in [29, 111] for unit-normal X).

PSUM budget: S^T double-buffer 2x3 banks + Y accumulator 1 + transpose 1.
"""

import sys

for _p in ("/opt/trn_rl_repo",):
    if _p not in sys.path:
        sys.path.insert(0, _p)

from contextlib import ExitStack

import numpy as np

import concourse.bass as bass
import concourse.tile as tile
from concourse import bacc, mybir
from concourse import bass_utils
from concourse.masks import make_identity

B, S, D = 8, 4096, 64
SHIFT = 32.0
QB = 512  # queries per block
JC = 128  # keys per chunk
GROUP = 3  # key chunks per exp group (PSUM banks per S^T buffer)
N_JC = S // JC  # 32
N_QB = S // QB  # 8

F32 = mybir.dt.float32
F32R = mybir.dt.float32r
BF16 = mybir.dt.bfloat16


def _body(ctx: ExitStack, tc: tile.TileContext, out: bass.AP, x: bass.AP):
    nc = tc.nc

    singles = ctx.enter_context(tc.tile_pool(name="singles", bufs=1))
    pt_pool = ctx.enter_context(tc.tile_pool(name="pt", bufs=3))
    ysb_pool = ctx.enter_context(tc.tile_pool(name="ysb", bufs=2))
    yout_pool = ctx.enter_context(tc.tile_pool(name="yout", bufs=4))
    st_ps = ctx.enter_context(tc.tile_pool(name="st", bufs=2, space="PSUM"))
    yacc_ps = ctx.enter_context(tc.tile_pool(name="yacc", bufs=1, space="PSUM"))
    ytr_ps = ctx.enter_context(tc.tile_pool(name="ytr", bufs=1, space="PSUM"))

    idf32 = singles.tile([D + 1, D + 1], F32)
    make_identity(nc, idf32)
    idbf = singles.tile([128, 128], BF16)
    make_identity(nc, idbf)

    bias = singles.tile([128, 1], F32)
    nc.vector.memset(bias, -SHIFT)

    xext = singles.tile([128, N_JC, D + 1], F32R)
    ones = singles.tile([128, N_JC], F32)
    nc.vector.memset(ones, 1.0)
    nc.vector.tensor_copy(xext[:, :, D], ones)

    xtdup = singles.tile([128, S], BF16)

    # Input phase: 4 chunks per unit. Per-unit DMAs alternate between the
    # sync and gpsimd DGE rings so loads land in parallel. Each slab is
    # converted to f32r (X_ext) and bf16, then transposed with paired PE
    # transposes (col groups 0/64) into a PSUM tile and copied into XTdup.
    # The transpose tiles rotate 3-deep through the ytr, yacc, and one st
    # slot, all idle until the steady-state pipeline starts.
    xld_pool = ctx.enter_context(tc.tile_pool(name="xld", bufs=8))

    def emit_input_unit(u):
        xld = xld_pool.tile([128, 4, D], F32, tag="xld", name="xld")
        src = x[u * 512 : (u + 1) * 512, :].rearrange("(c p) d -> p c d", p=128)
        nc.sync.dma_start(xld, src)
        nc.vector.tensor_copy(xext[:, 4 * u : 4 * u + 4, 0:D], xld)
        xbf = xld_pool.tile([128, 4, D], BF16, tag="xbf", name="xbf")
        nc.vector.tensor_copy(xbf, xld)
        pool, tag = [(ytr_ps, "ytr"), (st_ps, "st")][u % 2]
        ptr = pool.tile([128, 4, 128], BF16, tag=tag, name="ptr")
        for c in range(4):
            nc.tensor.transpose(
                ptr[0:64, c, :], xbf[:, c, :], idbf, tile_position=(0, 0)
            )
            nc.tensor.transpose(
                ptr[64:128, c, :], xbf[:, c, :], idbf, tile_position=(0, 64)
            )
        nc.vector.tensor_copy(
            xtdup[:, u * 512 : (u + 1) * 512].rearrange("p (c j) -> p c j", c=4), ptr
        )

    # Global flattened group schedule.
    groups = []  # (qb, [chunks])
    for qb in range(N_QB):
        lo = 0
        while lo < N_JC:
            groups.append((qb, list(range(lo, min(lo + GROUP, N_JC)))))
            lo += GROUP
    n_g = len(groups)

    def emit_st(i):
        qb, chunks = groups[i]
        st = st_ps.tile([128, GROUP, QB], F32, tag="st")
        q0 = qb * QB
        for ci, jc in enumerate(chunks):
            half = jc % 2
            rows = slice(64 * half, 64 * half + 64)
            nc.tensor.matmul(
                st[:, ci, :],
                xtdup[rows, jc * JC : (jc + 1) * JC],
                xtdup[rows, q0 : q0 + QB],
                start=True,
                stop=True,
                tile_position=(64 * half, 0),
            )
        return st

    def emit_exp(st, i):
        w = len(groups[i][1])
        pt = pt_pool.tile([128, GROUP, QB], F32R, tag="pt")
        nc.scalar.activation(
            pt[:, 0:w, :],
            st[:, 0:w, :],
            mybir.ActivationFunctionType.Exp,
            bias=bias,
            scale=1.0,
        )
        return pt

    def emit_pv(i, pt, yacc):
        for ci, jc in enumerate(groups[i][1]):
            nc.tensor.matmul(
                yacc,
                xext[:, jc, :],
                pt[:, ci, :],
                start=(jc == 0),
                stop=(jc == N_JC - 1),
            )

    def emit_epilogue(qb, yacc):
        ysb = ysb_pool.tile([D + 1, QB], F32, tag="ysb")
        for c in range(QB // 128):
            cs = slice(c * 128, (c + 1) * 128)
            nc.vector.tensor_copy(ysb[:, cs], yacc[:, cs])
            ytr = ytr_ps.tile([128, D + 1], F32, tag="ytr", name="ytr")
            nc.tensor.transpose(ytr, ysb[:, cs], idf32)
            rinv = yout_pool.tile([128, 1], F32, tag="rinv")
            nc.vector.reciprocal(rinv, ytr[:, D : D + 1])
            yo = yout_pool.tile([128, D], F32, tag="yo")
            nc.vector.tensor_scalar_mul(yo, ytr[:, 0:D], rinv)
            eng = nc.sync if c % 2 == 0 else nc.gpsimd
            eng.dma_start(out[qb * QB + c * 128 : qb * QB + (c + 1) * 128, :], yo)

    n_units = N_JC // 4
    units_emitted = 0

    def ensure_units(n):
        nonlocal units_emitted
        while units_emitted < min(n, n_units):
            emit_input_unit(units_emitted)
            units_emitted += 1

    def units_needed(i):
        qb, chunks = groups[i]
        hi = max(chunks[-1], (qb + 1) * (QB // JC) - 1)
        return hi // 4 + 1

    ensure_units(n_units)
    st_tiles = {0: emit_st(0), 1: emit_st(1)}
    yaccs = {}
    for i in range(n_g):
        qb, chunks = groups[i]
        if qb not in yaccs:
            yaccs[qb] = yacc_ps.tile([D + 1, QB], F32, tag="yacc", name="yacc")
        pt = emit_exp(st_tiles.pop(i), i)
        if i + 2 < n_g:
            st_tiles[i + 2] = emit_st(i + 2)
        emit_pv(i, pt, yaccs[qb])
        if chunks[-1] == N_JC - 1:
            emit_epilogue(qb, yaccs.pop(qb))


def build():
    nc = bacc.Bacc("TRN2", target_bir_lowering=False, debug=False, num_devices=B)
    x = nc.dram_tensor("X", (S, D), F32, kind="ExternalInput").ap()
    out = nc.dram_tensor("out", (S, D), F32, kind="ExternalOutput").ap()
    with tile.TileContext(nc) as tc:
        with ExitStack() as ctx:
            _body(ctx, tc, out, x)
    nc.compile()
    return nc


_NC = None


def run(X: np.ndarray, trace: bool = False, tmpdir: str | None = None):
    global _NC
    if _NC is None:
        _NC = build()
    X = np.asarray(X, dtype=np.float32)
    in_maps = [{"X": np.ascontiguousarray(X[b])} for b in range(B)]
    res = bass_utils.run_bass_kernel_spmd(
        _NC, in_maps, core_ids=list(range(B)), trace=trace, tmpdir=tmpdir
    )
    out = np.stack([res.results[b]["out"] for b in range(B)], axis=0).astype(np.float32)
    return out, res


def kernel(X: np.ndarray) -> np.ndarray:
    out, _ = run(X, trace=False)
    return out


# revision 17
# speedup vs baseline: 1.0583x; 1.0525x over previous
"""Fused self-attention kernel for Trainium2 (8 NeuronCores, batch-parallel).

Computes, for X of shape (8, 4096, 64):
    out[b] = softmax(X[b] @ X[b].T, axis=-1) @ X[b]
with one batch per NeuronCore.

Per-core algorithm (flash-style, fully on-chip):
  - XTdup (128, 4096) bf16: X^T replicated on partition halves 0-63 and
    64-127 (built with paired PE transposes into col groups 0/64), so the
    K=64 S^T matmuls can be row-packed pairwise via tile_position and run
    two-at-a-time on the PE array.
  - X_ext (4096, 65) = [X | ones] in float32r (PV stationary operand).
  - Per 512-query block, in groups of 3 key-chunks (128 keys each):
      S^T chunks = XT[keys].T @ XT[:, queries]     (bf16, PSUM 3 banks)
      P^T = exp(S^T - 32)                          (one 1536-wide ACTIVATE)
      Y^T_ext += X_ext[keys].T @ P^T               (f32r, PSUM-accumulated
                                                    across the whole block)
    The ones column of X_ext makes row 64 the softmax denominator.
  - PE transposes Y^T_ext back, DVE divides by the denominator, DMA out.
  The group pipeline is flattened across query blocks: S^T emission runs
  two groups ahead of exp/PV so ScalarE (the bottleneck) never starves.

softmax(S) == softmax(S - 32) exactly; the global shift keeps exp within
fp32 range (row maxima of S lie in [29, 111] for unit-normal X).

PSUM budget: S^T double-buffer 2x3 banks + Y accumulator 1 + transpose 1.
"""

import sys

for _p in ("/opt/trn_rl_repo",):
    if _p not in sys.path:
        sys.path.insert(0, _p)

from contextlib import ExitStack

import numpy as np

import concourse.bass as bass
import concourse.tile as tile
from concourse import bacc, mybir
from concourse import bass_utils
from concourse.masks import make_identity

B, S, D = 8, 4096, 64
SHIFT = 32.0
QB = 512  # queries per block
JC = 128  # keys per chunk
GROUP = 3  # key chunks per exp group (PSUM banks per S^T buffer)
N_JC = S // JC  # 32
N_QB = S // QB  # 8

F32 = mybir.dt.float32
F32R = mybir.dt.float32r
BF16 = mybir.dt.bfloat16


def _body(ctx: ExitStack, tc: tile.TileContext, out: bass.AP, x: bass.AP):
    nc = tc.nc

    singles = ctx.enter_context(tc.tile_pool(name="singles", bufs=1))
    pt_pool = ctx.enter_context(tc.tile_pool(name="pt", bufs=3))
    ysb_pool = ctx.enter_context(tc.tile_pool(name="ysb", bufs=2))
    yout_pool = ctx.enter_context(tc.tile_pool(name="yout", bufs=4))
    st_ps = ctx.enter_context(tc.tile_pool(name="st", bufs=2, space="PSUM"))
    yacc_ps = ctx.enter_context(tc.tile_pool(name="yacc", bufs=1, space="PSUM"))
    ytr_ps = ctx.enter_context(tc.tile_pool(name="ytr", bufs=1, space="PSUM"))

    idf32 = singles.tile([D + 1, D + 1], F32)
    make_identity(nc, idf32)
    idbf = singles.tile([128, 128], BF16)
    make_identity(nc, idbf)

    bias = singles.tile([128, 1], F32)
    nc.vector.memset(bias, -SHIFT)

    xext = singles.tile([128, N_JC, D + 1], F32R)
    ones = singles.tile([128, N_JC], F32)
    nc.vector.memset(ones, 1.0)
    nc.vector.tensor_copy(xext[:, :, D], ones)

    xtdup = singles.tile([128, S], BF16)

    # Input phase: 4 chunks per unit. Per-unit DMAs alternate between the
    # sync and gpsimd DGE rings so loads land in parallel. Each slab is
    # converted to f32r (X_ext) and bf16, then transposed with paired PE
    # transposes (col groups 0/64) into a PSUM tile and copied into XTdup.
    # The transpose tiles rotate 3-deep through the ytr, yacc, and one st
    # slot, all idle until the steady-state pipeline starts.
    xld_pool = ctx.enter_context(tc.tile_pool(name="xld", bufs=8))

    def emit_input_unit(u):
        xld = xld_pool.tile([128, 4, D], F32, tag="xld", name="xld")
        src = x[u * 512 : (u + 1) * 512, :].rearrange("(c p) d -> p c d", p=128)
        nc.sync.dma_start(xld, src)
        nc.vector.tensor_copy(xext[:, 4 * u : 4 * u + 4, 0:D], xld)
        xbf = xld_pool.tile([128, 4, D], BF16, tag="xbf", name="xbf")
        nc.vector.tensor_copy(xbf, xld)
        pool, tag = [(ytr_ps, "ytr"), (st_ps, "st")][u % 2]
        ptr = pool.tile([128, 4, 128], BF16, tag=tag, name="ptr")
        for c in range(4):
            nc.tensor.transpose(
                ptr[0:64, c, :], xbf[:, c, :], idbf, tile_position=(0, 0)
            )
            nc.tensor.transpose(
                ptr[64:128, c, :], xbf[:, c, :], idbf, tile_position=(0, 64)
            )
        nc.vector.tensor_copy(
            xtdup[:, u * 512 : (u + 1) * 512].rearrange("p (c j) -> p c j", c=4), ptr
        )

    # Global flattened group schedule.
    groups = []  # (qb, [chunks])
    for qb in range(N_QB):
        lo = 0
        while lo < N_JC:
            groups.append((qb, list(range(lo, min(lo + GROUP, N_JC)))))
            lo += GROUP
    n_g = len(groups)

    def emit_st(i):
        qb, chunks = groups[i]
        st = st_ps.tile([128, GROUP, QB], F32, tag="st")
        q0 = qb * QB
        for ci, jc in enumerate(chunks):
            half = jc % 2
            rows = slice(64 * half, 64 * half + 64)
            nc.tensor.matmul(
                st[:, ci, :],
                xtdup[rows, jc * JC : (jc + 1) * JC],
                xtdup[rows, q0 : q0 + QB],
                start=True,
                stop=True,
                tile_position=(64 * half, 0),
            )
        return st

    def emit_exp(st, i):
        w = len(groups[i][1])
        pt = pt_pool.tile([128, GROUP, QB], F32R, tag="pt")
        nc.scalar.activation(
            pt[:, 0:w, :],
            st[:, 0:w, :],
            mybir.ActivationFunctionType.Exp,
            bias=bias,
            scale=1.0,
        )
        return pt

    def emit_pv(i, pt, yacc):
        for ci, jc in enumerate(groups[i][1]):
            nc.tensor.matmul(
                yacc,
                xext[:, jc, :],
                pt[:, ci, :],
                start=(jc == 0),
                stop=(jc == N_JC - 1),
            )

    def emit_epilogue(qb, yacc):
        ysb = ysb_pool.tile([D + 1, QB], F32, tag="ysb")
        nc.vector.tensor_copy(ysb, yacc)
        for c in range(QB // 128):
            ytr = ytr_ps.tile([128, D + 1], F32, tag="ytr", name="ytr")
            nc.tensor.transpose(ytr, ysb[:, c * 128 : (c + 1) * 128], idf32)
            rinv = yout_pool.tile([128, 1], F32, tag="rinv")
            nc.vector.reciprocal(rinv, ytr[:, D : D + 1])
            yo = yout_pool.tile([128, D], F32, tag="yo")
            nc.vector.tensor_scalar_mul(yo, ytr[:, 0:D], rinv)
            nc.sync.dma_start(out[qb * QB + c * 128 : qb * QB + (c + 1) * 128, :], yo)

    n_units = N_JC // 4
    units_emitted = 0

    def ensure_units(n):
        nonlocal units_emitted
        while units_emitted < min(n, n_units):
            emit_input_unit(units_emitted)
            units_emitted += 1

    def units_needed(i):
        qb, chunks = groups[i]
        hi = max(chunks[-1], (qb + 1) * (QB // JC) - 1)
        return hi // 4 + 1

    ensure_units(n_units)
    st_tiles = {0: emit_st(0), 1: emit_st(1)}
    yaccs = {}
    for i in range(n_g):
        qb, chunks = groups[i]
        if qb not in yaccs:
            yaccs[qb] = yacc_ps.tile([D + 1, QB], F32, tag="yacc", name="yacc")
        pt = emit_exp(st_tiles.pop(i), i)
        if i + 2 < n_g:
            st_tiles[i + 2] = emit_st(i + 2)
        emit_pv(i, pt, yaccs[qb])
        if chunks[-1] == N_JC - 1:
            emit_epilogue(qb, yaccs.pop(qb))


def build():
    nc = bacc.Bacc("TRN2", target_bir_lowering=False, debug=False, num_devices=B)
    x = nc.dram_tensor("X", (S, D), F32, kind="ExternalInput").ap()
    out = nc.dram_tensor("out", (S, D), F32, kind="ExternalOutput").ap()
    with tile.TileContext(nc) as tc:
        with ExitStack() as ctx:
            _body(ctx, tc, out, x)
    nc.compile()
    return nc


_NC = None


def run(X: np.ndarray, trace: bool = False, tmpdir: str | None = None):
    global _NC
    if _NC is None:
        _NC = build()
    X = np.asarray(X, dtype=np.float32)
    in_maps = [{"X": np.ascontiguousarray(X[b])} for b in range(B)]
    res = bass_utils.run_bass_kernel_spmd(
        _NC, in_maps, core_ids=list(range(B)), trace=trace, tmpdir=tmpdir
    )
    out = np.stack([res.results[b]["out"] for b in range(B)], axis=0).astype(np.float32)
    return out, res


def kernel(X: np.ndarray) -> np.ndarray:
    out, _ = run(X, trace=False)
    return out
